# revision 1
# baseline (speedup 1.0000x reference)
"""Navier-Stokes PINN kernel for 8x Trainium2 NeuronCores.

Math: the reference MLP uses ReLU activations, so the network is piecewise
linear in its inputs. All second and third derivatives produced by jax AD are
exactly zero; the PDE residuals collapse to
    u = dpsi/dy,  v = -dpsi/dx,  p = MLP(z)[1],  f = dp/dx,  g = dp/dy.
Everything is computable from one forward pass plus two forward-mode tangent
streams (d/dx, d/dy) through the masked linear layers:
    A_1 = z @ Win + b_in,        H_1 = relu(A_1),  M_1 = step(A_1)
    T_1k = row_k(Win),           G_1k = M_1 * T_1k          (k in {x, y})
    A_i = H_{i-1} @ W_i + b_i,   T_ik = G_{i-1,k} @ W_i
    H_i = relu(A_i), M_i = step(A_i), G_ik = M_i * T_ik
    [u v p f g] = [H_L ; G_Lx ; G_Ly] @ Wfin + bfin
with Wfin assembled on the host from Wout columns (including the -1 sign
for v), so the 5 outputs come out of one accumulated matmul.

Layout: feature-major on chip — activations are (features, points) tiles so
every layer is lhsT.T @ rhs with lhsT = weight block, rhs = activations.
Sharding: pure data parallel, 8192 points per core, weights replicated.

Matmul dtype: float32r (single-pass reduced-precision fp32) runs 4x faster
than float32 on the PE. Walrus requires every f32r matmul operand to be
produced by an instruction that rounds to f32r, so compute producers write
through f32r-bitcast APs and DMA-landed tensors get one-time round-copies.

Default mode "c2" (see _build_c2): ReLU masks need ~fp32-accurate
pre-activations (pure-f32r flips masks near zero -> rel err 2.7e-2), so the
forward pass carries every tensor as an f32r (hi, res) pair and accumulates
A = Whi@Hhi + Wres@Hhi + Whi@Hres in PSUM. The first two terms are f32r
(1 cycle/row); the Whi@Hres correction (~2^-12 of A) tolerates fp8 and both
k-halves collapse into ONE fp8e5m2 DoubleRow matmul at 0.5 cycles/row.
f32r keeps ~12 mantissa bits, so residuals (~2^-13 of H) would underflow
e5m2's 2^-16 denormal floor: compensated-layer H tiles are carried scaled
by 2^7 (folded free into the relu scale) with forward weights pre-scaled by
2^-7, keeping every PSUM term at true scale. The input layer packs its three
K=3 compensation terms into one K=9 matmul against a stacked [zhi;zhi;zres].
Tangent masking fuses into one DVE op per half-layer via
scalar_tensor_tensor: G = (relu > 0) * T. Engine split per half-layer:
scalar relu+round-copies, DVE fused tangent mask-mult + one residual,
Pool the other residual; weights arrive as 4 packed DMAs (HWDGE costs
~625ns per descriptor) and per-engine scheduler priorities stage late-layer
weights below the software-pipelined next-block input package.
Measured: rel err 5.7e-3 (gate 2e-2), cost-model timeline 309us vs 392us
for the fp32-forward baseline.
"""

import os

import numpy as np

NCORES = 8
N_TOTAL = 65536
NPC = N_TOTAL // NCORES  # points per core
HID = 256
NL = 4  # hidden->hidden layers (L=6 total: in + 4 hidden + out)
B = 512  # points per block
NB = NPC // B
P = 128
NH = HID // P  # feature halves

# matmul precision: "r" = float32r (fast, reduced precision),
# "f" = float32 (exact, 4x slower), "c" = legacy compensated,
# "c2" = compensated forward with rebalanced engine assignment.
FWD_MODE = os.environ.get("NS_FWD_MODE", "c2")
TAN_MODE = os.environ.get("NS_TAN_MODE", "r")
POOL_OFFLOAD = os.environ.get("NS_POOL", "0") == "1"
TSPLIT = os.environ.get("NS_TSPLIT", "0") == "1"
# repeat the whole workload R times inside one NEFF (timing harness only)
REPEAT = int(os.environ.get("NS_REPEAT", "1"))

_NC_CACHE = {}


def _build(fwd_mode: str, tan_mode: str):
    import concourse.tile as tile
    from concourse import bacc, mybir

    f32 = mybir.dt.float32
    f32r = mybir.dt.float32r
    Relu = mybir.ActivationFunctionType.Relu
    Ident = mybir.ActivationFunctionType.Identity
    Copy = mybir.ActivationFunctionType.Copy
    mult = mybir.AluOpType.mult

    def rnd(ap, mode):
        return ap.bitcast(f32r) if mode == "r" else ap

    nc = bacc.Bacc(
        "TRN2",
        target_bir_lowering=False,
        debug=False,
        enable_asserts=False,
        num_devices=NCORES,
    )

    zt_d = nc.dram_tensor("zt", (3, NPC), f32, kind="ExternalInput").ap()
    win_d = nc.dram_tensor("win", (3, HID), f32, kind="ExternalInput").ap()
    wint_d = nc.dram_tensor("wint", (HID, 2), f32, kind="ExternalInput").ap()
    bin_d = nc.dram_tensor("bin", (HID, 1), f32, kind="ExternalInput").ap()
    wh_d = nc.dram_tensor("wh", (NL, HID, HID), f32, kind="ExternalInput").ap()
    bh_d = nc.dram_tensor("bh", (NL, HID, 1), f32, kind="ExternalInput").ap()
    wfin_d = nc.dram_tensor("wfin", (3 * HID, 5), f32, kind="ExternalInput").ap()
    bfin_d = nc.dram_tensor("bfin", (5, 1), f32, kind="ExternalInput").ap()
    out_d = nc.dram_tensor("out", (5, NPC), f32, kind="ExternalOutput").ap()

    need_r = fwd_mode in ("r", "c") or tan_mode == "r"

    with tile.TileContext(nc) as tc:
        with (
            tc.tile_pool(name="weights", bufs=1) as wpool,
            tc.tile_pool(name="zin", bufs=6) as zpool,
            tc.tile_pool(name="acts", bufs=4) as hpool,
            tc.tile_pool(name="tans", bufs=4) as gpool,
            tc.tile_pool(name="masks", bufs=4) as mpool,
            tc.tile_pool(name="outs", bufs=6) as opool,
            tc.tile_pool(name="psA", bufs=3, space="PSUM") as psA,
            tc.tile_pool(name="psT", bufs=4 if TSPLIT else 2, space="PSUM") as psT,
            tc.tile_pool(name="psO", bufs=1, space="PSUM") as psO,
        ):
            # ---- one-time weight staging ----
            def stage(name, shape, src_ap, rounded, resid=False):
                """DMA a weight into SBUF; optionally add an f32r round-copy
                and (for the compensated forward) a rounded residual W - Whi.

                Returns (f32_tile, rounded_tile, residual_tile).
                """
                t = wpool.tile(shape, f32, tag=name, name=name)
                nc.sync.dma_start(t[:], src_ap)
                if not rounded:
                    return t, None, None
                tr = wpool.tile(shape, f32, tag=name + "r", name=name + "r")
                nc.scalar.activation(tr[:].bitcast(f32r), t[:], Copy)
                if not resid:
                    return t, tr, None
                ts_ = wpool.tile(shape, f32, tag=name + "s", name=name + "s")
                nc.vector.tensor_tensor(
                    ts_[:].bitcast(f32r), t[:], tr[:], mybir.AluOpType.subtract
                )
                return t, tr, ts_

            def pick(trip, mode):
                t, tr, _ = trip
                return tr[:].bitcast(f32r) if mode == "r" else t[:]

            win_t = stage("win", [3, HID], win_d[:, :], fwd_mode == "r")
            # compensated forward: hidden weights need rounded + residual parts
            wh_resid = fwd_mode == "c"
            wint_t = []
            bin_t = []
            for h in range(NH):
                w = wpool.tile([P, 2], f32, tag=f"wint{h}", name=f"wint{h}")
                nc.sync.dma_start(w[:], wint_d[h * P : (h + 1) * P, :])
                wint_t.append(w)
                b = wpool.tile([P, 1], f32, tag=f"bin{h}", name=f"bin{h}")
                nc.sync.dma_start(b[:], bin_d[h * P : (h + 1) * P, :])
                bin_t.append(b)
            wh_t = {}
            bh_t = {}
            for li in range(NL):
                for k in range(NH):
                    for h in range(NH):
                        wh_t[li, k, h] = stage(
                            f"wh{li}{k}{h}",
                            [P, P],
                            wh_d[li, k * P : (k + 1) * P, h * P : (h + 1) * P],
                            need_r,
                            resid=wh_resid,
                        )
                for h in range(NH):
                    b = wpool.tile([P, 1], f32, tag=f"bh{li}{h}", name=f"bh{li}{h}")
                    nc.sync.dma_start(b[:], bh_d[li, h * P : (h + 1) * P, :])
                    bh_t[li, h] = b
            wfin_t = []
            for k in range(3 * NH):
                wfin_t.append(
                    stage(
                        f"wfin{k}",
                        [P, 5],
                        wfin_d[k * P : (k + 1) * P, :],
                        need_r,
                    )
                )
            bfin_t = wpool.tile([5, 1], f32, tag="bfin", name="bfin")
            nc.sync.dma_start(bfin_t[:], bfin_d[:, :])

            # ---- per-block pipeline ----
            sub = mybir.AluOpType.subtract

            def make_h(a, bias_ap, li, h):
                """relu + mask from PSUM A; returns (h_for_mm, mask_tile).

                In compensated mode the forward consumes an (hhi, hres) f32r
                pair; the last hidden H is rounded-only (feeds the f32r
                output matmul, no masks downstream of it to protect).
                """
                h_mode = tan_mode if li == NL - 1 else fwd_mode
                ht = hpool.tile([P, B], f32, tag=f"H{h}", name=f"H{h}")
                mt = mpool.tile([P, B], f32, tag=f"M{h}", name=f"M{h}")
                if h_mode == "c":
                    nc.scalar.activation(ht[:], a[:], Relu, bias=bias_ap)
                    if h == 0:
                        nc.scalar.sign(mt[:], ht[:])
                    else:
                        nc.vector.tensor_scalar(
                            mt[:], ht[:], 0.0, None, mybir.AluOpType.is_gt
                        )
                    hhi = hpool.tile([P, B], f32, tag=f"Hh{h}", name=f"Hh{h}")
                    nc.scalar.activation(hhi[:].bitcast(f32r), ht[:], Copy)
                    hres = hpool.tile([P, B], f32, tag=f"Hs{h}", name=f"Hs{h}")
                    eng = nc.gpsimd if POOL_OFFLOAD else nc.vector
                    eng.tensor_tensor(hres[:].bitcast(f32r), ht[:], hhi[:], sub)
                    return (hhi, hres), mt
                nc.scalar.activation(rnd(ht[:], h_mode), a[:], Relu, bias=bias_ap)
                nc.scalar.sign(mt[:], ht[:])
                return ht, mt

            for ib_rep in range(NB * REPEAT):
                ib = ib_rep % NB
                zt = zpool.tile([3, B], f32, tag="zt", name="zt")
                # gpsimd queue: don't serialize behind the weight-stage DMAs
                nc.gpsimd.dma_start(zt[:], zt_d[:, ib * B : (ib + 1) * B])
                if fwd_mode == "r":
                    zr = zpool.tile([3, B], f32, tag="zr", name="zr")
                    nc.scalar.activation(zr[:].bitcast(f32r), zt[:], Copy)
                    z_mm = zr[:].bitcast(f32r)
                else:
                    z_mm = zt[:]

                # input layer: A1 = Win.T @ z (fp32 in modes f/c), relu+mask,
                # tangent init
                Hs, Gs = [], []
                for h in range(NH):
                    a = psA.tile([P, B], f32, tag="A", name="A")
                    nc.tensor.matmul(
                        a[:],
                        pick(win_t, fwd_mode)[:, h * P : (h + 1) * P],
                        z_mm,
                        start=True,
                        stop=True,
                    )
                    hmm, mt = make_h(a, bin_t[h][:, 0:1], -1, h)
                    gt = gpool.tile([P, 2 * B], f32, tag=f"G{h}", name=f"G{h}")
                    ieng = nc.gpsimd if POOL_OFFLOAD else nc.vector
                    ieng.tensor_scalar(
                        rnd(gt[:, 0:B], tan_mode), mt[:], wint_t[h][:, 0:1], None, mult
                    )
                    ieng.tensor_scalar(
                        rnd(gt[:, B : 2 * B], tan_mode),
                        mt[:],
                        wint_t[h][:, 1:2],
                        None,
                        mult,
                    )
                    Hs.append(hmm)
                    Gs.append(gt)

                # hidden layers
                for li in range(NL):
                    nHs, nGs = [], []
                    for h in range(NH):
                        a = psA.tile([P, B], f32, tag="A", name="A")
                        if fwd_mode == "c":
                            n_mm = 3 * NH
                            i_mm = 0
                            for k in range(NH):
                                whi = wh_t[li, k, h][1][:].bitcast(f32r)
                                wres = wh_t[li, k, h][2][:].bitcast(f32r)
                                hhi = Hs[k][0][:].bitcast(f32r)
                                hres = Hs[k][1][:].bitcast(f32r)
                                for lhs, rhs in (
                                    (whi, hhi),
                                    (whi, hres),
                                    (wres, hhi),
                                ):
                                    nc.tensor.matmul(
                                        a[:],
                                        lhs,
                                        rhs,
                                        start=(i_mm == 0),
                                        stop=(i_mm == n_mm - 1),
                                    )
                                    i_mm += 1
                        else:
                            for k in range(NH):
                                nc.tensor.matmul(
                                    a[:],
                                    pick(wh_t[li, k, h], fwd_mode),
                                    rnd(Hs[k][:], fwd_mode),
                                    start=(k == 0),
                                    stop=(k == NH - 1),
                                )
                        hmm, mt = make_h(a, bh_t[li, h][:, 0:1], li, h)
                        gt = gpool.tile([P, 2 * B], f32, tag=f"G{h}", name=f"G{h}")
                        if TSPLIT:
                            for d in range(2):
                                tp1 = psT.tile([P, B], f32, tag="T", name="T")
                                for k in range(NH):
                                    nc.tensor.matmul(
                                        tp1[:],
                                        pick(wh_t[li, k, h], tan_mode),
                                        rnd(Gs[k][:, d * B : (d + 1) * B], tan_mode),
                                        start=(k == 0),
                                        stop=(k == NH - 1),
                                    )
                                nc.vector.tensor_tensor(
                                    rnd(gt[:, d * B : (d + 1) * B], tan_mode),
                                    tp1[:],
                                    mt[:],
                                    mult,
                                )
                        else:
                            tps = psT.tile([P, 2 * B], f32, tag="T", name="T")
                            for d in range(2):
                                for k in range(NH):
                                    nc.tensor.matmul(
                                        tps[:, d * B : (d + 1) * B],
                                        pick(wh_t[li, k, h], tan_mode),
                                        rnd(Gs[k][:, d * B : (d + 1) * B], tan_mode),
                                        start=(k == 0),
                                        stop=(k == NH - 1),
                                    )
                            m3 = mt[:].unsqueeze(1).broadcast_to((P, 2, B))
                            nc.vector.tensor_tensor(
                                rnd(gt[:], tan_mode).rearrange("p (d b) -> p d b", d=2),
                                tps[:].rearrange("p (d b) -> p d b", d=2),
                                m3,
                                mult,
                            )
                        nHs.append(hmm)
                        nGs.append(gt)
                    Hs, Gs = nHs, nGs

                # output layer: [H ; Gx ; Gy] @ Wfin -> (5, B)
                ops = psO.tile([5, B], f32, tag="O", name="O")
                chunks = [
                    (rnd(Hs[0][:], tan_mode), tan_mode),
                    (rnd(Hs[1][:], tan_mode), tan_mode),
                ]
                for d in range(2):
                    for h in range(NH):
                        chunks.append(
                            (rnd(Gs[h][:, d * B : (d + 1) * B], tan_mode), tan_mode)
                        )
                for k in range(6):
                    rhs_ap, mode = chunks[k]
                    nc.tensor.matmul(
                        ops[:],
                        pick(wfin_t[k], mode),
                        rhs_ap,
                        start=(k == 0),
                        stop=(k == 5),
                    )
                osb = opool.tile([5, B], f32, tag="osb", name="osb")
                nc.scalar.activation(osb[:], ops[:], Ident, bias=bfin_t[:, 0:1])
                nc.gpsimd.dma_start(out_d[:, ib * B : (ib + 1) * B], osb[:])

    nc.compile()
    return nc


def _build_c2():
    """Compensated-forward kernel, engine-rebalanced.

    All matmuls are f32r (1 cycle/row vs fp32's 4). Mask fidelity comes from
    compensation: every f32r-rounded tensor X is carried as a (hi, res) pair
    with hi = round_f32r(X), res = X - hi, and each forward matmul accumulates
    whi@hhi + whi@hres + wres@hhi in PSUM (the res@res term is ~u^2, dropped).
    Measured on HW this keeps rel err ~1e-3 while pure f32r masks give 2.7e-2.

    Engine split per half-layer (PE is the intended bottleneck at ~19.6us
    per 512-point block; each elementwise [128,512] op costs ~0.55-0.66us on
    scalar/DVE and ~0.8-1.1us on Pool):
      scalar (Activation): relu from PSUM (+bias), hi round-copies
      DVE: masks from PSUM via is_gt(add(A, bias), 0), tangent mask-mult
      Pool (gpsimd): residual subtracts, input-layer tangent init, z DMA
    GPSIMD cannot read PSUM (walrus verifier), so everything Pool touches
    is SBUF-resident.
    """
    import concourse.tile as tile
    from concourse import bacc, mybir

    f32 = mybir.dt.float32
    f32r = mybir.dt.float32r
    f8e5 = mybir.dt.float8e5
    DoubleRow = mybir.MatmulPerfMode.DoubleRow
    Relu = mybir.ActivationFunctionType.Relu
    Ident = mybir.ActivationFunctionType.Identity
    Copy = mybir.ActivationFunctionType.Copy
    mult = mybir.AluOpType.mult
    add = mybir.AluOpType.add
    sub = mybir.AluOpType.subtract
    is_gt = mybir.AluOpType.is_gt

    nc = bacc.Bacc(
        "TRN2",
        target_bir_lowering=False,
        debug=False,
        enable_asserts=False,
        num_devices=NCORES,
    )

    # Packed inputs (assembled host-side in kernel()): HWDGE costs ~625ns
    # per DMA descriptor, so the 36 small weight/bias transfers of the naive
    # layout serialize ~22us of startup. Packed: 4 descriptors.
    #   whp:  (P, NL*NH*HID)  wh[li][k*P+p][c] at [p, ((li*NH)+k)*HID + c]
    #   bp:   (P, 15)  cols 0-1 bin halves; 2-9 bh[li][h]; 10-13 wint[k][d]
    #         at col 10+2*k+d; col 14 rows 0-4 = bfin
    #   wfp:  (P, 6*5)  wfin[cc*P+p][o] at [p, cc*5+o]
    zt_d = nc.dram_tensor("zt", (3, NPC), f32, kind="ExternalInput").ap()
    win_d = nc.dram_tensor("win", (3, HID), f32, kind="ExternalInput").ap()
    whp_d = nc.dram_tensor("whp", (P, NL * NH * HID), f32, kind="ExternalInput").ap()
    bp_d = nc.dram_tensor("bp", (P, 25), f32, kind="ExternalInput").ap()
    wfp_d = nc.dram_tensor("wfp", (P, 6 * 5), f32, kind="ExternalInput").ap()
    out_d = nc.dram_tensor("out", (5, NPC), f32, kind="ExternalOutput").ap()

    with tile.TileContext(nc) as tc:
        with (
            tc.tile_pool(name="weights", bufs=1) as wpool,
            tc.tile_pool(name="zin", bufs=6) as zpool,
            tc.tile_pool(name="acts", bufs=4) as hpool,
            tc.tile_pool(name="tans", bufs=4) as gpool,
            tc.tile_pool(name="masks", bufs=4) as mpool,
            tc.tile_pool(name="outs", bufs=6) as opool,
            tc.tile_pool(name="psA", bufs=3, space="PSUM") as psA,
            tc.tile_pool(name="psT", bufs=2, space="PSUM") as psT,
            tc.tile_pool(name="psO", bufs=1, space="PSUM") as psO,
        ):
            # ---- one-time weight staging (packed: 4 DMA descriptors) ----
            win_t = wpool.tile([3, HID], f32, tag="win", name="win")
            nc.sync.dma_start(win_t[:], win_d[:, :])
            bp_t = wpool.tile([P, 25], f32, tag="bp", name="bp")
            nc.sync.dma_start(bp_t[:], bp_d[:, :])
            whp_t = wpool.tile([P, NL * NH * HID], f32, tag="whp", name="whp")
            nc.sync.dma_start(whp_t[:], whp_d[:, :])

            def bias(col):
                return bp_t[:, col : col + 1]

            bin_t = [bias(h) for h in range(NH)]
            bh_b = {(li, h): bias(2 + 2 * li + h) for li in range(NL) for h in range(NH)}
            wint_b = {(k, d): bias(10 + 2 * k + d) for k in range(NH) for d in range(2)}
            bfin_t = bp_t[0:5, 14:15]
            # Compensated-layer H tiles are carried scaled by 2^7 (folded
            # into the relu's scale) so the f32r residuals (~2^-13 of H)
            # land mid-range in fp8e5m2 instead of under its 2^-16 denormal
            # floor; the forward weights are pre-scaled by 2^-7 at staging
            # so every A-term lands at true scale in PSUM. Cols 15-16 hold
            # 128*bin and 17-24 hold 128*bh for those relus.
            bin_s = [bias(15 + h) for h in range(NH)]
            bh_s = {
                (li, h): bias(17 + 2 * li + h) for li in range(NL) for h in range(NH)
            }

            # Combined input lhsT [winhi; winres; winhi] (K=9): the three
            # compensation terms of A1 = Win.T z collapse into ONE matmul
            # against a stacked rhs [zhi; zhi; zres] since K is tiny.
            # compute engines may only access partition-0-aligned ranges;
            # build the K=9 stack with aligned writes + SBUF-to-SBUF DMA
            winc = wpool.tile([9, HID], f32, tag="winc", name="winc")
            nc.scalar.activation(winc[0:3, :].bitcast(f32r), win_t[:], Copy)
            wres3 = wpool.tile([3, HID], f32, tag="wres3", name="wres3")
            nc.vector.tensor_tensor(
                wres3[:].bitcast(f32r), win_t[:], winc[0:3, :], sub
            )
            nc.sync.dma_start(winc[3:6, :], wres3[:])
            nc.sync.dma_start(winc[6:9, :], winc[0:3, :])

            # hi/res pairs per layer, derived from the packed tile with one
            # whole-layer round-copy + subtract each. Layer-0 derives keep
            # early priority (block 0 needs them within ~6us); later layers
            # yield to block 0's input/L0 elementwise chain.
            wh_hi = {}
            wh_hiS = {}
            wh_resS = {}
            for li in range(NL):
                ctx = tc.high_priority(offset=-(260 + 80 * li)) if li > 0 else None
                if ctx is not None:
                    ctx.__enter__()
                raw = whp_t[:, li * NH * HID : (li + 1) * NH * HID]
                hi = wpool.tile([P, NH * HID], f32, tag=f"whh{li}", name=f"whh{li}")
                nc.scalar.activation(hi[:].bitcast(f32r), raw, Copy)
                hiS = wpool.tile([P, NH * HID], f32, tag=f"whhS{li}", name=f"whhS{li}")
                nc.scalar.activation(hiS[:].bitcast(f32r), raw, Copy, scale=1.0 / 128)
                resS = wpool.tile([P, NH * HID], f32, tag=f"whsS{li}", name=f"whsS{li}")
                nc.vector.scalar_tensor_tensor(
                    resS[:].bitcast(f32r), raw, 1.0 / 128, hiS[:], mult, sub
                )
                if ctx is not None:
                    ctx.__exit__(None, None, None)
                wh_hi[li] = hi
                wh_hiS[li] = hiS
                wh_resS[li] = resS

            # e5m2 interleaved weights [whi(k0); whi(k1)] per (li, h) for
            # the DoubleRow residual-correction matmul: one half-rate fp8
            # matmul computes sum_k whi(k)@hres(k). The correction term is
            # ~2^-12 of A, so 2 mantissa bits on the weights cost only
            # ~0.17*2^-12 of A -- about 3x the f32r PE epsilon.
            whiE5 = {}
            for li in range(NL):
                ctx = tc.high_priority(offset=-(260 + 80 * li)) if li > 0 else None
                if ctx is not None:
                    ctx.__enter__()
                for h in range(NH):
                    w8 = wpool.tile(
                        [P, NH, P], f8e5, tag=f"whE{li}{h}", name=f"whE{li}{h}"
                    )
                    for k in range(NH):
                        nc.scalar.activation(
                            w8[:, k, :],
                            whp_t[:, (li * NH + k) * HID + h * P : (li * NH + k) * HID + (h + 1) * P],
                            Copy,
                            scale=1.0 / 128,
                        )
                    whiE5[li, h] = w8
                if ctx is not None:
                    ctx.__exit__(None, None, None)

            def wh_slice(tile_, k, h):
                return tile_[:, k * HID + h * P : k * HID + (h + 1) * P]

            # (scaled-hi, scaled-res) for the forward A terms; unscaled hi
            # for the tangent matmuls (the G stream is true-scale).
            wh_t = {
                (li, k, h): (wh_slice(wh_hiS[li], k, h), wh_slice(wh_resS[li], k, h))
                for li in range(NL)
                for k in range(NH)
                for h in range(NH)
            }
            wh_tan = {
                (li, k, h): wh_slice(wh_hi[li], k, h)
                for li in range(NL)
                for k in range(NH)
                for h in range(NH)
            }
            bh_t = bh_b

            wfin_t = []
            with tc.high_priority(offset=-300):
                wfp_t = wpool.tile([P, 6 * 5], f32, tag="wfp", name="wfp")
                nc.sync.dma_start(wfp_t[:], wfp_d[:, :])
                wfr = wpool.tile([P, 6 * 5], f32, tag="wfr", name="wfr")
                nc.scalar.activation(wfr[:].bitcast(f32r), wfp_t[:], Copy)
                for cc in range(6):
                    wfin_t.append(wfr[:, cc * 5 : (cc + 1) * 5])

            # L0 tangent weights pre-scaled by the input-tangent seed:
            # T(L0)[f,b] = sum_j W0[j,f]*wint_d[j]*M[j,b], so fold wint into
            # the weight rows once at staging and feed the masks directly as
            # the T(L0) rhs. This removes the per-block Ginit multiplies
            # (4 Pool ops) and one rounding from the tangent path.
            wl0 = {}
            with tc.high_priority(offset=-120):
              for d in range(2):
                  for k in range(NH):
                      wsc = wpool.tile(
                          [P, HID], f32, tag=f"wl0{d}{k}", name=f"wl0{d}{k}"
                      )
                      nc.vector.tensor_scalar(
                          wsc[:].bitcast(f32r),
                          whp_t[:, k * HID : (k + 1) * HID],
                          wint_b[k, d],
                          None,
                          mult,
                      )
                      for h in range(NH):
                          wl0[d, k, h] = wsc[:, h * P : (h + 1) * P]

            # ---- per-block pipeline (2-stage software pipeline) ----
            # The input stage of block ib+1 is issued BEFORE the hidden
            # stage of block ib, so its serial z->A1->relu->hi/res chain
            # (~3.5us crossing four engines) completes while the PE chews
            # block ib's hidden layers; without this the PE stalls ~2us at
            # every block boundary waiting for the input-layer residuals.
            # Input-stage tiles use dedicated tags ("in" suffix): they stay
            # live until the NEXT block's hidden stage, and sharing a ring
            # with the hidden-layer tiles would create circular waits.
            def input_stage(ib, first=False):
                with tc.high_priority(offset=-250):
                    zt = zpool.tile([3, B], f32, tag="zt", name="zt", bufs=3)
                    nc.scalar.dma_start(zt[:], zt_d[:, ib * B : (ib + 1) * B])
                    zc = zpool.tile([9, B], f32, tag="zc", name="zc", bufs=3)
                    nc.gpsimd.tensor_scalar(
                        zc[0:3, :].bitcast(f32r), zt[:], 0.0, None, add
                    )
                    zres = zpool.tile([3, B], f32, tag="zres", name="zres", bufs=3)
                    nc.gpsimd.tensor_tensor(
                        zres[:].bitcast(f32r), zt[:], zc[0:3, :], sub
                    )
                    if not first:
                        nc.scalar.dma_start(zc[3:6, :], zc[0:3, :])
                        nc.scalar.dma_start(zc[6:9, :], zres[:])

                # input layer: compensated A1 = Win.T @ z in ONE K=9 matmul
                # per half against the stacked [zhi; zhi; zres]. Block 0
                # instead uses three K=3 matmuls straight from the unstacked
                # tiles: the SBUF-to-SBUF stacking DMAs cost ~2.5us each in
                # completion latency, which only the first block would eat.
                Hs, Ms = [], []
                a_t = []
                for h in range(NH):
                    a = psA.tile([P, B], f32, tag="A", name="A")
                    cols = slice(h * P, (h + 1) * P)
                    if first:
                        for i_mm, (lhs, rhs) in enumerate(
                            (
                                (winc[0:3, cols], zc[0:3, :]),
                                (wres3[:, cols], zc[0:3, :]),
                                (winc[0:3, cols], zres[:]),
                            )
                        ):
                            nc.tensor.matmul(
                                a[:],
                                lhs.bitcast(f32r),
                                rhs.bitcast(f32r),
                                start=(i_mm == 0),
                                stop=(i_mm == 2),
                            )
                    else:
                        nc.tensor.matmul(
                            a[:],
                            winc[:, cols].bitcast(f32r),
                            zc[:].bitcast(f32r),
                            start=True,
                            stop=True,
                        )
                    a_t.append(a)
                # The elementwise package runs at a LATE priority: this stage
                # is injected mid-block (after L0 of the previous block), and
                # its ops become ready before that block's L1+ relus do. The
                # scheduler is a ready-heap keyed on priority, so without the
                # demotion these ops win the scalar/DVE engines and stall the
                # hidden-layer residual chain (~1.3us/block on the PE).
                hres2 = hpool.tile([P, NH, B], f8e5, tag="Hsin2", name="Hsin2", bufs=2)
                with tc.high_priority(offset=-250):
                    for h in range(NH):
                        a = a_t[h]
                        ht = hpool.tile([P, B], f32, tag=f"Hin{h}", name=f"Hin{h}", bufs=2)
                        nc.scalar.activation(
                            ht[:], a[:], Relu, bias=bin_s[h], scale=128.0
                        )
                        # mask via scalar sign(relu): on DVE this op delays
                        # the L0 G-mults (it becomes ready first), stalling
                        # T(L1) ~0.6us/block
                        mt = mpool.tile([P, B], f32, tag=f"Min{h}", name=f"Min{h}", bufs=2)
                        nc.scalar.sign(mt[:].bitcast(f32r), ht[:])
                        hhi = hpool.tile([P, B], f32, tag=f"Hhin{h}", name=f"Hhin{h}", bufs=2)
                        nc.scalar.activation(hhi[:].bitcast(f32r), ht[:], Copy)
                        eng = nc.gpsimd if h == 0 else nc.vector
                        eng.tensor_tensor(hres2[:, h, :], ht[:], hhi[:], sub)
                        Hs.append(hhi)
                        Ms.append(mt)
                return (Hs, hres2), Ms

            def rest_of_block(ib, HsIn, Ms, inject=None):
                Hs, hres2 = HsIn
                # hidden layers. Issue order is tuned for the in-order
                # engine queues: both halves' A matmuls go first (their
                # operands — prev layer's hi/res — are ready earliest), then
                # both halves' T matmuls (their operands — prev G — come off
                # DVE last); elementwise follows readiness order per engine.
                # `inject` (the next block's input stage) is called after
                # layer 0 so its scalar/Pool package lands in the queues
                # behind L0's relu/hi/res (which feed the critical L0->L1
                # chain) and its PE matmuls pad the T(L0)->A(L1) window;
                # the input-stage Ginit multiplies are further deferred to
                # after L2 (they are Pool-heavy and not needed until the
                # next block's T(L0)).
                inj_Hs = inj_Ms = None
                Gs = None
                for li in range(NL):
                    last = li == NL - 1
                    nHs, nGs = [], []
                    a_t, tps_t, ht_t, mt_t = [], [], [], []
                    for h in range(NH):
                        a = psA.tile([P, B], f32, tag="A", name="A")
                        # hi-operand f32r terms first; the residual
                        # correction is ONE fp8e5m2 DoubleRow matmul
                        # (sum_k whi(k)@hres(k) at half rate) issued last —
                        # the hres tiles are the latest-arriving operands,
                        # and PSUM accumulation order is free.
                        terms = []
                        for k in range(NH):
                            whi, wres = wh_t[li, k, h]
                            terms.append((whi, Hs[k][:]))
                            terms.append((wres, Hs[k][:]))
                        for i_mm, (lhs, rhs) in enumerate(terms):
                            nc.tensor.matmul(
                                a[:],
                                lhs.bitcast(f32r),
                                rhs.bitcast(f32r),
                                start=(i_mm == 0),
                                stop=False,
                            )
                        nc.tensor.matmul(
                            a[:],
                            whiE5[li, h][:],
                            hres2[:],
                            start=False,
                            stop=True,
                            perf_mode=DoubleRow,
                        )
                        a_t.append(a)
                    for h in range(NH):
                        tps = psT.tile([P, 2 * B], f32, tag="T", name="T")
                        for d in range(2):
                            for k in range(NH):
                                if li == 0:
                                    lhs = wl0[d, k, h]
                                    rhs = Ms[k][:]
                                else:
                                    lhs = wh_tan[li, k, h]
                                    rhs = Gs[k][:, d * B : (d + 1) * B]
                                nc.tensor.matmul(
                                    tps[:, d * B : (d + 1) * B],
                                    lhs.bitcast(f32r),
                                    rhs.bitcast(f32r),
                                    start=(k == 0),
                                    stop=(k == NH - 1),
                                )
                        tps_t.append(tps)
                    nres2 = None
                    if not last:
                        nres2 = hpool.tile(
                            [P, NH, B], f8e5, tag="Hs2", name="Hs2", bufs=3
                        )
                    for h in range(NH):
                        a = a_t[h]
                        ht = hpool.tile([P, B], f32, tag=f"H{h}", name=f"H{h}", bufs=3)
                        ht_out = ht[:].bitcast(f32r) if last else ht[:]
                        if last:
                            nc.scalar.activation(ht_out, a[:], Relu, bias=bh_t[li, h])
                        else:
                            nc.scalar.activation(
                                ht_out, a[:], Relu, bias=bh_s[li, h], scale=128.0
                            )
                        ht_t.append(ht)
                        if last:
                            nHs.append(ht)
                        else:
                            hhi = hpool.tile([P, B], f32, tag=f"Hh{h}", name=f"Hh{h}", bufs=3)
                            nc.scalar.activation(hhi[:].bitcast(f32r), ht[:], Copy)
                            # h0 residual on Pool (ready earlier, slower
                            # engine), h1 on DVE (ready later, faster): the
                            # consuming DoubleRow needs BOTH halves
                            eng = nc.gpsimd if h == 0 else nc.vector
                            eng.tensor_tensor(nres2[:, h, :], ht[:], hhi[:], sub)
                            nHs.append(hhi)
                    for h in range(NH):
                        # fused mask+multiply: G = (relu(A+b) > 0) * T in one
                        # DVE op. The mask stage reads ht from SBUF (walrus
                        # allows only ONE PSUM input per DVE instruction, and
                        # T comes from PSUM); is_gt(relu(A+b), 0) == step(A+b)
                        # exactly, f32r rounding preserves sign.
                        gt = gpool.tile([P, 2 * B], f32, tag=f"G{h}", name=f"G{h}", bufs=3)
                        h3 = ht_t[h][:].unsqueeze(1).broadcast_to((P, 2, B))
                        nc.vector.scalar_tensor_tensor(
                            gt[:].bitcast(f32r).rearrange("p (d b) -> p d b", d=2),
                            h3,
                            0.0,
                            tps_t[h][:].rearrange("p (d b) -> p d b", d=2),
                            is_gt,
                            mult,
                        )
                        nGs.append(gt)
                    Hs, Gs, hres2 = nHs, nGs, nres2
                    if li == 0 and inject is not None:
                        inj_Hs, inj_Ms = inject()

                # output layer: [H ; Gx ; Gy] @ Wfin -> (5, B)
                ops = psO.tile([5, B], f32, tag="O", name="O")
                chunks = [Hs[0][:].bitcast(f32r), Hs[1][:].bitcast(f32r)]
                for d in range(2):
                    for h in range(NH):
                        chunks.append(
                            Gs[h][:, d * B : (d + 1) * B].bitcast(f32r)
                        )
                for k in range(6):
                    nc.tensor.matmul(
                        ops[:],
                        wfin_t[k].bitcast(f32r),
                        chunks[k],
                        start=(k == 0),
                        stop=(k == 5),
                    )
                osb = opool.tile([5, B], f32, tag="osb", name="osb", bufs=4)
                nc.scalar.activation(osb[:], ops[:], Ident, bias=bfin_t)
                nc.scalar.dma_start(out_d[:, ib * B : (ib + 1) * B], osb[:])
                return (inj_Hs, inj_Ms) if inj_Hs is not None else None

            n_blocks = NB * REPEAT
            cur = input_stage(0, first=True)
            for idx in range(n_blocks):
                inj = None
                if idx + 1 < n_blocks:
                    nib = (idx + 1) % NB
                    inj = lambda nib=nib: input_stage(nib)  # noqa: E731
                cur = rest_of_block(idx % NB, *cur, inject=inj)

    nc.compile()
    return nc


def _get_nc():
    key = (FWD_MODE, TAN_MODE, REPEAT, POOL_OFFLOAD, TSPLIT)
    if key not in _NC_CACHE:
        if FWD_MODE == "c2":
            _NC_CACHE[key] = _build_c2()
        else:
            _NC_CACHE[key] = _build(FWD_MODE, TAN_MODE)
    return _NC_CACHE[key]


def kernel(x, y, t, Win, b_in, Wh, b_h, Wout, b_out, _trace=False):
    from concourse import bass_utils

    x = np.asarray(x, np.float32)
    y = np.asarray(y, np.float32)
    t = np.asarray(t, np.float32)
    Win = np.asarray(Win, np.float32)
    b_in = np.asarray(b_in, np.float32)
    Wh = np.asarray(Wh, np.float32)
    b_h = np.asarray(b_h, np.float32)
    Wout = np.asarray(Wout, np.float32)
    b_out = np.asarray(b_out, np.float32)

    z = np.ascontiguousarray(
        np.stack([x[:, 0], y[:, 0], t[:, 0]], axis=0)
    )  # (3, N)
    wfin = np.zeros((3 * HID, 5), np.float32)
    wfin[2 * HID : 3 * HID, 0] = Wout[:, 0]  # u = dpsi/dy
    wfin[HID : 2 * HID, 1] = -Wout[:, 0]  # v = -dpsi/dx
    wfin[0:HID, 2] = Wout[:, 1]  # p
    wfin[HID : 2 * HID, 3] = Wout[:, 1]  # f = dp/dx
    wfin[2 * HID : 3 * HID, 4] = Wout[:, 1]  # g = dp/dy

    nc = _get_nc()
    if FWD_MODE == "c2":
        # packed weight layouts (see _build_c2): few large DMAs instead of
        # many small ones — HWDGE costs ~625ns per descriptor.
        whp = np.ascontiguousarray(
            Wh.reshape(NL, NH, P, HID).transpose(2, 0, 1, 3).reshape(P, NL * NH * HID)
        )
        bp = np.zeros((P, 25), np.float32)
        bp[:, 0:2] = b_in.reshape(NH, P).T
        bp[:, 2:10] = b_h.reshape(NL, NH, P).transpose(2, 0, 1).reshape(P, NL * NH)
        for k in range(NH):
            for d in range(2):
                bp[:, 10 + 2 * k + d] = Win[d, k * P : (k + 1) * P]
        bp[2, 14] = b_out[1]  # bfin: only the p output has a bias
        # scaled biases for the 2^7-scaled compensated-layer relus
        bp[:, 15:17] = 128.0 * bp[:, 0:2]
        bp[:, 17:25] = 128.0 * bp[:, 2:10]
        wfp = np.ascontiguousarray(
            wfin.reshape(6, P, 5).transpose(1, 0, 2).reshape(P, 30)
        )
        in_maps = []
        for c in range(NCORES):
            in_maps.append(
                {
                    "zt": np.ascontiguousarray(z[:, c * NPC : (c + 1) * NPC]),
                    "win": Win,
                    "whp": whp,
                    "bp": bp,
                    "wfp": wfp,
                }
            )
        res = bass_utils.run_bass_kernel_spmd(
            nc, in_maps, core_ids=list(range(NCORES)), trace=_trace
        )
        kernel._last_results = res
        full = np.concatenate(
            [res.results[c]["out"] for c in range(NCORES)], axis=1
        )  # (5, N)
        return np.ascontiguousarray(full[:, :, None].astype(np.float32))

    wint = np.ascontiguousarray(Win[0:2, :].T)  # (HID, 2)
    binc = np.ascontiguousarray(b_in.reshape(HID, 1))
    bhc = np.ascontiguousarray(b_h.reshape(NL, HID, 1))
    bfin = np.zeros((5, 1), np.float32)
    bfin[2, 0] = b_out[1]
    in_maps = []
    for c in range(NCORES):
        in_maps.append(
            {
                "zt": np.ascontiguousarray(z[:, c * NPC : (c + 1) * NPC]),
                "win": Win,
                "wint": wint,
                "bin": binc,
                "wh": Wh,
                "bh": bhc,
                "wfin": wfin,
                "bfin": bfin,
            }
        )
    res = bass_utils.run_bass_kernel_spmd(
        nc, in_maps, core_ids=list(range(NCORES)), trace=_trace
    )
    kernel._last_results = res
    full = np.concatenate(
        [res.results[c]["out"] for c in range(NCORES)], axis=1
    )  # (5, N)
    return np.ascontiguousarray(full[:, :, None].astype(np.float32))

